# revision 7
# baseline (speedup 1.0000x reference)
"""DARTS recurrent cell (T=70, B=256, D=850) on 8 TRN2 NeuronCores.

Data-parallel over batch (32 rows/core). Per core:
  phase 1: xw0[t] = x[t] @ W0x  for all t (weights streamed once)
  phase 2: 70 recurrent steps; 6 of 9 weight matrices resident in SBUF
           (fp16), {W0h, Ws2, Ws5} rotate through one shared stream slot.
Matmuls in fp16 (PSUM accumulates fp32): emulated end-to-end error vs
fp32 reference ~2e-4 (scale-relative).
"""
import sys

sys.path.insert(0, "/opt/trn_rl_repo")

import numpy as np

import concourse.bacc as bacc
import concourse.bass as bass
import concourse.mybir as mybir
from concourse.bass_utils import run_bass_kernel_spmd
from concourse.tile import TileContext

T, B, D = 70, 256, 850
NC = 8
BL = B // NC          # 32 batch rows per core
KT = 7                # ceil(850/128) contraction tiles
KP = [128] * 6 + [D - 6 * 128]   # last tile 82 rows
D2 = 2 * D

RECURRENT = [('tanh', 0), ('relu', 1), ('relu', 1), ('identity', 1),
             ('tanh', 2), ('sigmoid', 5), ('tanh', 3), ('relu', 5)]

# step order: edge index sequence honoring the DAG, with streamed
# matrices (W0h first, Ws2 fourth, Ws5 eighth) well spaced for the
# shared stream slot rotation.
EDGE_ORDER = [0, 1, 2, 3, 4, 6, 5, 7]
STREAMED = (2, 5)      # edge weights streamed through the shared slot
RESIDENT = [0, 1, 3, 4, 6, 7]

F16 = mybir.dt.float16
F32 = mybir.dt.float32
AF = mybir.ActivationFunctionType


def _pack_w(w):
    """[850, N] fp -> [KT, 128, N] fp16 with zero padding of rows."""
    n = w.shape[1]
    out = np.zeros((KT, 128, n), np.float16)
    for k in range(KT):
        r0, r1 = 128 * k, min(128 * (k + 1), D)
        out[k, : r1 - r0] = w[r0:r1].astype(np.float16)
    return out


def _build(nsteps):
    nc = bacc.Bacc("TRN2", target_bir_lowering=False, debug=False,
                   num_devices=NC)

    xT_d = nc.dram_tensor("xT", [nsteps, 128, KT * BL], F16,
                          kind="ExternalInput")
    w0x_d = nc.dram_tensor("w0x", [KT, 128, D2], F16, kind="ExternalInput")
    w0h_d = nc.dram_tensor("w0h", [KT, 128, D2], F16, kind="ExternalInput")
    ws_d = [nc.dram_tensor(f"ws{i}", [KT, 128, D2], F16,
                           kind="ExternalInput") for i in range(8)]
    hT0_d = nc.dram_tensor("hT0", [128, KT * BL], F16, kind="ExternalInput")
    hbt0_d = nc.dram_tensor("hbt0", [BL, D], F16, kind="ExternalInput")
    ident_d = nc.dram_tensor("ident", [32, 32], F16, kind="ExternalInput")
    xw0_d = nc.dram_tensor("xw0", [nsteps, BL, D2], F16)     # scratch
    out_d = nc.dram_tensor("hid", [nsteps, BL, D], F32, kind="ExternalOutput")

    with TileContext(nc) as tc:
        with tc.tile_pool(name="wres", bufs=1) as wres_p, \
             tc.tile_pool(name="wstream", bufs=1) as wstr_p, \
             tc.tile_pool(name="xt", bufs=2) as xt_p, \
             tc.tile_pool(name="xw", bufs=2) as xw_p, \
             tc.tile_pool(name="sbt", bufs=7) as sbt_p, \
             tc.tile_pool(name="st", bufs=6) as st_p, \
             tc.tile_pool(name="acc", bufs=1) as acc_p, \
             tc.tile_pool(name="hout", bufs=2) as hout_p, \
             tc.tile_pool(name="cst", bufs=1) as cst_p, \
             tc.tile_pool(name="sig", bufs=2) as sig_p, \
             tc.tile_pool(name="ps", bufs=2, space="PSUM") as ps_p:

            ident = cst_p.tile([32, 32], F16, tag="ident")
            nc.gpsimd.dma_start(ident[:], ident_d[:, :])

            # resident edge weights: one [128, KT*D2] tile per matrix
            wres = {}
            for i in RESIDENT:
                wt = wres_p.tile([128, KT * D2], F16, tag=f"w{i}")
                nc.gpsimd.dma_start(
                    wt[:].rearrange("p (k n) -> p k n", k=KT),
                    ws_d[i].rearrange("k p n -> p k n"))
                wres[i] = wt

            def stream(dram):
                wt = wstr_p.tile([128, KT * D2], F16, tag="wslot")
                nc.gpsimd.dma_start(
                    wt[:].rearrange("p (k n) -> p k n", k=KT),
                    dram.rearrange("k p n -> p k n"))
                return wt

            def mm_group(stat, w_sb):
                """[BL, D2] = stat.T @ W  (stat: [128, KT*BL] fp16 k-tiles)"""
                ps = ps_p.tile([BL, 2048], F32, tag="ps")
                for half in (0, 1):
                    for c0, c1 in ((0, 512), (512, D)):
                        for k in range(KT):
                            kp = KP[k]
                            nc.tensor.matmul(
                                ps[:, 1024 * half + c0:1024 * half + c1],
                                stat[:kp, k * BL:(k + 1) * BL],
                                w_sb[:kp, k * D2 + half * D + c0:
                                     k * D2 + half * D + c1],
                                start=(k == 0), stop=(k == KT - 1))
                return ps[:, 0:D], ps[:, 1024:1024 + D]

            def transpose_to(s16):
                """[BL, D] fp16 -> [128, KT*BL] fp16 (zero-padded tail)."""
                sT = st_p.tile([128, KT * BL], F16, tag="sT")
                for k in range(KT):
                    kp = KP[k]
                    pst = ps_p.tile([128, BL], F16, tag="ps")
                    nc.tensor.transpose(
                        pst[:kp, :], s16[:, 128 * k:128 * k + kp], ident[:, :])
                    if kp < 128:
                        nc.vector.memset(sT[:, k * BL:(k + 1) * BL], 0.0)
                    nc.vector.tensor_copy(sT[:kp, k * BL:(k + 1) * BL],
                                          pst[:kp, :])
                return sT

            # ---------------- phase 1: xw0[t] = x[t] @ W0x ----------------
            w0x_sb = stream(w0x_d)
            for t in range(nsteps):
                xt = xt_p.tile([128, KT * BL], F16, tag="xt")
                nc.gpsimd.dma_start(xt[:], xT_d[t])
                psc, psh = mm_group(xt, w0x_sb)
                xo = xw_p.tile([BL, D2], F16, tag="xwo")
                nc.vector.tensor_copy(xo[:, :D], psc[:])
                nc.vector.tensor_copy(xo[:, D:], psh[:])
                nc.gpsimd.dma_start(xw0_d[t], xo[:])

            # ---------------- phase 2: recurrence ----------------
            hT = cst_p.tile([128, KT * BL], F16, tag="hT0t")
            nc.gpsimd.dma_start(hT[:], hT0_d[:, :])
            h16 = sbt_p.tile([BL, D], F16, tag="sbt")
            nc.gpsimd.dma_start(h16[:], hbt0_d[:, :])

            for t in range(nsteps):
                xw = xw_p.tile([BL, D2], F16, tag="xwi")
                nc.gpsimd.dma_start(xw[:], xw0_d[t])

                w0h_sb = stream(w0h_d)
                psc, psh = mm_group(hT, w0h_sb)
                nc.vector.tensor_add(psc[:], psc[:], xw[:, :D])
                nc.vector.tensor_add(psh[:], psh[:], xw[:, D:])

                # s0 = h + sig(c0) * (tanh(h0) - h)
                sig = sig_p.tile([BL, D], F16, tag="sig")
                nc.scalar.activation(sig[:], psc[:], AF.Sigmoid)
                nc.scalar.activation(psh[:], psh[:], AF.Tanh)
                nc.vector.tensor_sub(psh[:], psh[:], h16[:])
                nc.vector.tensor_mul(psh[:], psh[:], sig[:])
                s16 = sbt_p.tile([BL, D], F16, tag="sbt")
                nc.vector.tensor_add(s16[:], psh[:], h16[:])

                states16 = [s16]
                statesT = {0: transpose_to(s16)}
                acc = acc_p.tile([BL, D], F32, tag="acc")
                first = True

                for j, ei in enumerate(EDGE_ORDER):
                    act, pred = RECURRENT[ei]
                    if ei in STREAMED:
                        w_sb = stream(ws_d[ei])
                    else:
                        w_sb = wres[ei]
                    sp16 = states16[pred]
                    psc, psh = mm_group(statesT[pred], w_sb)
                    sig = sig_p.tile([BL, D], F16, tag="sig")
                    nc.scalar.activation(sig[:], psc[:], AF.Sigmoid)
                    if act == 'tanh':
                        nc.scalar.activation(psh[:], psh[:], AF.Tanh)
                    elif act == 'sigmoid':
                        nc.scalar.activation(psh[:], psh[:], AF.Sigmoid)
                    elif act == 'relu':
                        nc.vector.tensor_relu(psh[:], psh[:])
                    # s = sp + sig * (act(h) - sp)
                    nc.vector.tensor_sub(psh[:], psh[:], sp16[:])
                    nc.vector.tensor_mul(psh[:], psh[:], sig[:])
                    s16 = sbt_p.tile([BL, D], F16, tag="sbt")
                    nc.vector.tensor_add(s16[:], psh[:], sp16[:])
                    while len(states16) <= ei + 1:
                        states16.append(None)
                    states16[ei + 1] = s16
                    # states consumed later as matmul inputs need transpose
                    if ei + 1 in (1, 2, 3, 5):
                        statesT[ei + 1] = transpose_to(s16)
                    if first:
                        nc.vector.tensor_copy(acc[:], s16[:])
                        first = False
                    else:
                        nc.vector.tensor_add(acc[:], acc[:], s16[:])

                ho = hout_p.tile([BL, D], F32, tag="ho")
                nc.vector.tensor_scalar_mul(ho[:], acc[:], 0.125)
                nc.gpsimd.dma_start(out_d[t], ho[:])
                h16 = sbt_p.tile([BL, D], F16, tag="sbt")
                nc.vector.tensor_scalar_mul(h16[:], acc[:], 0.125)
                hT = transpose_to(h16)

    nc.finalize()
    return nc


_NC_CACHE = {}


def make_in_maps(inputs, hidden, W0, Ws):
    nsteps = inputs.shape[0]
    w0x_p = _pack_w(W0[:D].astype(np.float32))
    w0h_p = _pack_w(W0[D:].astype(np.float32))
    ws_p = [_pack_w(np.asarray(Ws)[i].astype(np.float32)) for i in range(8)]
    ident = np.eye(32, dtype=np.float16)

    in_maps = []
    for c in range(NC):
        xs = np.asarray(inputs)[:, c * BL:(c + 1) * BL, :].astype(np.float16)
        # xT[t, p, k*BL + b] = x[t, b, 128k + p]
        xT = np.zeros((nsteps, 128, KT * BL), np.float16)
        xsw = np.swapaxes(xs, 1, 2)          # [t, 850, BL]
        for k in range(KT):
            kp = KP[k]
            xT[:, :kp, k * BL:(k + 1) * BL] = xsw[:, 128 * k:128 * k + kp, :]
        hb = np.asarray(hidden)[0, c * BL:(c + 1) * BL, :].astype(np.float16)
        hT0 = np.zeros((128, KT * BL), np.float16)
        hsw = hb.T                            # [850, BL]
        for k in range(KT):
            kp = KP[k]
            hT0[:kp, k * BL:(k + 1) * BL] = hsw[128 * k:128 * k + kp, :]
        m = {"xT": xT, "w0x": w0x_p, "w0h": w0h_p, "hT0": hT0,
             "hbt0": hb, "ident": ident}
        for i in range(8):
            m[f"ws{i}"] = ws_p[i]
        in_maps.append(m)
    return in_maps


def kernel(inputs, hidden, W0, Ws):
    nsteps = np.asarray(inputs).shape[0]
    if nsteps not in _NC_CACHE:
        _NC_CACHE[nsteps] = _build(nsteps)
    nc = _NC_CACHE[nsteps]
    in_maps = make_in_maps(inputs, hidden, W0, Ws)
    res = run_bass_kernel_spmd(nc, in_maps, core_ids=list(range(NC)))
    hid = np.concatenate([res.results[c]["hid"] for c in range(NC)], axis=1)
    hid = np.ascontiguousarray(hid.astype(np.float32))
    return hid, hid[-1][None]
